# revision 18
# baseline (speedup 1.0000x reference)
"""2D Haar DWT (pywt.dwt2 'haar') on Trainium2, sharded across 8 NeuronCores.

Full input x: [8192, 8192] f32. Output: [4, 4096, 4096] f32 (cA, cH, cV, cD).

Sharding: row-wise. Core i handles rows [1024*i, 1024*(i+1)), producing output
rows [512*i, 512*(i+1)) of every subband. 2x2 haar blocks never cross the
chunk boundary, so no halo exchange.

Per-core dataflow (default "fullstore" layout; per 256-row block, 4 per core):
  - one 8MB DMA loads the block's 128 even rows into the left half and 128 odd
    rows into the right half of a [128, 2*8192] SBUF tile -- the row
    deinterleave is free in the DMA access pattern and every descriptor moves
    32KB contiguous
  - ScalarE: e *= 0.5 in place
  - VectorE scalar_tensor_tensor: d = (o * -0.5) + e = 0.5*(even-odd), into e
  - VectorE tensor_add:           s = d + o         = 0.5*(even+odd), into o
  - VectorE tensor_add/sub on stride-2 column views finish the butterfly,
    writing full-width subband pairs (cA|cH then cV|cD) into two alternating
    [128, 2*4096] tiles:
      cA = s[2j]+s[2j+1], cH = d[2j]+d[2j+1], cV = s[2j]-s[2j+1], cD = d[2j]-d[2j+1]
  - two 4MB DMAs store the pairs with 16KB-contiguous descriptors, issued on
    the ScalarE HWDGE ring so stores never head-of-line-block loads on the SP
    ring

The kernel is DMA-bound: 64MB of HBM traffic per core at ~340-350 GB/s
effective (~193us measured via repeat-NEFF differencing; the pure-roofline
floor at 358 GB/s/core is ~179us). Larger descriptor runs measurably beat
finer-grained pipelining on real HW (chunked 8KB-run variant: 243us).
"""

import numpy as np

H = 8192
W = 8192
NCORES = 8
HC = H // NCORES  # 1024 rows per core
P = 128  # partitions
C = 4096  # column chunk width (input cols per tile)
CH = C // 2  # output cols per tile per subband
N_RB = HC // (2 * P)  # 4 row blocks (each covers 256 input rows)
N_CC = W // C  # 2 column chunks

_CACHE: dict = {}


def _build_nc(
    repeat: int = 1,
    store_engine: str = "scalar",
    in_bufs: int = 2,
    s_bufs: int = 4,
    out_bufs: int = 1,
    scale_engine: str = "scalar",
    chunk: int = 4096,
    load_engine: str = "sync",
    mode: str = "full",
    layout: str = "fullstore",
    stage2_split: bool = False,
    combined_load: bool = True,
):
    import concourse.bacc as bacc
    import concourse.mybir as mybir
    from concourse.tile import TileContext

    f32 = mybir.dt.float32
    Alu = mybir.AluOpType

    nc = bacc.Bacc("TRN2", target_bir_lowering=False, debug=False)
    x = nc.dram_tensor("x", [HC, W], f32, kind="ExternalInput").ap()
    out = nc.dram_tensor("out", [4, HC // 2, W // 2], f32, kind="ExternalOutput").ap()

    CC = chunk
    CCH = CC // 2
    n_cc = W // CC
    # x rows: rb*256 + p*2 + eo ; cols: cc*CC + c
    xr = x.rearrange("(rb p eo) (cc c) -> rb cc p eo c", p=P, eo=2, cc=n_cc)
    # out: subband s, row rb*128 + p, col cc*CCH + c
    outr = out.rearrange("s (rb p) (cc c) -> rb cc p s c", p=P, c=CCH)

    if layout == "fullstore":
        # Full-width everything: one combined [128, 2W] load per row block
        # (32KB runs), full-width stage-2, and per-subband-pair full-width
        # stores (16KB runs). Output double-buffered via two alternating
        # 2-subband pools so SBUF fits: 128 + 32 + 32 = 192KB.
        CW = W // 2
        xr3 = x.rearrange("(rb p eo) w -> rb p eo w", p=P, eo=2)
        xr2f = x.rearrange("(rb p eo) w -> rb eo p w", p=P, eo=2)
        # out dims for a 2-subband store: [p, s(2), c(W/2)]
        outp = out.rearrange("(sp s) (rb p) c -> rb sp p s c", s=2, p=P)
        with TileContext(nc) as tc:
            with (
                tc.tile_pool(name="inp", bufs=in_bufs) as in_pool,
                tc.tile_pool(name="onp", bufs=in_bufs) as o_pool_f,
                tc.tile_pool(name="outa", bufs=out_bufs) as pool_a,
                tc.tile_pool(name="outb", bufs=out_bufs) as pool_b,
            ):
                for _rep in range(repeat):
                    for rb in range(N_RB):
                        if combined_load:
                            in_t = in_pool.tile([P, 2 * W], f32)
                            getattr(nc, load_engine).dma_start(
                                out=in_t.rearrange("p (eo w) -> p eo w", eo=2),
                                in_=xr3[rb],
                            )
                            e_t = in_t[:, 0:W]
                            o_t = in_t[:, W : 2 * W]
                        else:
                            e_t = in_pool.tile([P, W], f32)
                            o_t = o_pool_f.tile([P, W], f32)
                            getattr(nc, load_engine).dma_start(out=e_t, in_=xr2f[rb, 0])
                            getattr(nc, load_engine).dma_start(out=o_t, in_=xr2f[rb, 1])
                        if scale_engine == "scalar":
                            nc.scalar.mul(e_t, e_t, 0.5)
                        else:
                            nc.gpsimd.tensor_scalar_mul(e_t, e_t, 0.5)
                        # d = -0.5*o + 0.5*e (into e half); s = d + o (into o half)
                        nc.vector.scalar_tensor_tensor(
                            out=e_t, in0=o_t, scalar=-0.5, in1=e_t,
                            op0=Alu.mult, op1=Alu.add,
                        )
                        nc.vector.tensor_add(o_t, e_t, o_t)
                        d_t, s_t2 = e_t, o_t
                        se = s_t2[:, 0:W:2]
                        so = s_t2[:, 1:W:2]
                        de = d_t[:, 0:W:2]
                        do = d_t[:, 1:W:2]
                        # pair 0: cA | cH ; pair 1: cV | cD
                        t_a = pool_a.tile([P, 2 * CW], f32)
                        t_b = pool_b.tile([P, 2 * CW], f32)
                        nc.vector.tensor_add(t_a[:, 0:CW], se, so)  # cA
                        nc.vector.tensor_add(t_a[:, CW : 2 * CW], de, do)  # cH
                        getattr(nc, store_engine).dma_start(
                            out=outp[rb, 0],
                            in_=t_a.rearrange("p (s c) -> p s c", s=2),
                        )
                        nc.vector.tensor_sub(t_b[:, 0:CW], se, so)  # cV
                        nc.vector.tensor_sub(t_b[:, CW : 2 * CW], de, do)  # cD
                        getattr(nc, store_engine).dma_start(
                            out=outp[rb, 1],
                            in_=t_b.rearrange("p (s c) -> p s c", s=2),
                        )
        nc.compile()
        return nc

    if layout == "fullrow":
        # Full-width loads (32KB contiguous per partition-row), stage-1 in
        # place (d over e, s over o), half-width stores.
        NSC = W // 2 // CCH  # store chunks per row block
        xr2 = x.rearrange("(rb p eo) w -> rb eo p w", p=P, eo=2)
        xr3 = x.rearrange("(rb p eo) w -> rb p eo w", p=P, eo=2)
        with TileContext(nc) as tc:
            with (
                tc.tile_pool(name="ep", bufs=in_bufs) as e_pool,
                tc.tile_pool(name="op", bufs=in_bufs) as o_pool,
                tc.tile_pool(name="outp", bufs=out_bufs) as out_pool,
            ):
                for _rep in range(repeat):
                    for rb in range(N_RB):
                        if combined_load:
                            in_t = e_pool.tile([P, 2 * W], f32)
                            getattr(nc, load_engine).dma_start(
                                out=in_t.rearrange("p (eo w) -> p eo w", eo=2),
                                in_=xr3[rb],
                            )
                            e_t = in_t[:, 0:W]
                            o_t = in_t[:, W : 2 * W]
                        else:
                            e_t = e_pool.tile([P, W], f32)
                            o_t = o_pool.tile([P, W], f32)
                            getattr(nc, load_engine).dma_start(out=e_t, in_=xr2[rb, 0])
                            getattr(nc, load_engine).dma_start(out=o_t, in_=xr2[rb, 1])
                        if mode != "dma":
                            if scale_engine == "scalar":
                                nc.scalar.mul(e_t, e_t, 0.5)
                            else:
                                nc.gpsimd.tensor_scalar_mul(e_t, e_t, 0.5)
                            # d = -0.5*o + 0.5*e  (into e_t)
                            nc.vector.scalar_tensor_tensor(
                                out=e_t, in0=o_t, scalar=-0.5, in1=e_t,
                                op0=Alu.mult, op1=Alu.add,
                            )
                            # s = d + o = 0.5*e + 0.5*o  (into o_t)
                            nc.vector.tensor_add(o_t, e_t, o_t)
                        d_t, s_t2 = e_t, o_t
                        for sc in range(NSC):
                            lo = sc * 2 * CCH
                            hi = (sc + 1) * 2 * CCH
                            out_t = out_pool.tile([P, 4 * CCH], f32)
                            if mode != "dma":
                                se = s_t2[:, lo:hi:2]
                                so = s_t2[:, lo + 1 : hi : 2]
                                de = d_t[:, lo:hi:2]
                                do = d_t[:, lo + 1 : hi : 2]
                                eng2 = nc.gpsimd if stage2_split else nc.vector
                                nc.vector.tensor_add(out_t[:, 0 * CCH : 1 * CCH], se, so)
                                eng2.tensor_add(out_t[:, 1 * CCH : 2 * CCH], de, do)
                                nc.vector.tensor_sub(out_t[:, 2 * CCH : 3 * CCH], se, so)
                                eng2.tensor_sub(out_t[:, 3 * CCH : 4 * CCH], de, do)
                                src_ap = out_t.rearrange("p (s c) -> p s c", s=4)
                            else:
                                src_ap = e_t[:, 0 : 4 * CCH].rearrange(
                                    "p (s c) -> p s c", s=4
                                )
                            getattr(nc, store_engine).dma_start(
                                out=outr[rb, sc], in_=src_ap
                            )
        nc.compile()
        return nc

    with TileContext(nc) as tc:
        with (
            tc.tile_pool(name="inp", bufs=in_bufs) as in_pool,
            tc.tile_pool(name="sum", bufs=s_bufs) as s_pool,
            tc.tile_pool(name="outp", bufs=out_bufs) as out_pool,
        ):
            for _rep in range(repeat):
                for rb in range(N_RB):
                    for cc in range(n_cc):
                        in_t = in_pool.tile([P, 2 * CC], f32)
                        if mode != "compute":
                            getattr(nc, load_engine).dma_start(
                                out=in_t.rearrange("p (eo c) -> p eo c", eo=2),
                                in_=xr[rb, cc],
                            )
                        if mode == "dma":
                            getattr(nc, store_engine).dma_start(
                                out=outr[rb, cc],
                                in_=in_t[:, 0 : 4 * CCH].rearrange(
                                    "p (s c) -> p s c", s=4
                                ),
                            )
                            continue
                        e = in_t[:, 0:CC]
                        o = in_t[:, CC : 2 * CC]
                        # e <- 0.5*e (off VectorE: ScalarE or GpSimd)
                        if scale_engine == "scalar":
                            nc.scalar.mul(e, e, 0.5)
                        else:
                            nc.gpsimd.tensor_scalar_mul(e, e, 0.5)
                        s_t = s_pool.tile([P, CC], f32)
                        # s = 0.5*o + e(=0.5e)  ;  d = -0.5*o + e  (d in place over o)
                        nc.vector.scalar_tensor_tensor(
                            out=s_t, in0=o, scalar=0.5, in1=e, op0=Alu.mult, op1=Alu.add
                        )
                        nc.vector.scalar_tensor_tensor(
                            out=o, in0=o, scalar=-0.5, in1=e, op0=Alu.mult, op1=Alu.add
                        )
                        se = s_t[:, 0:CC:2]
                        so = s_t[:, 1:CC:2]
                        de = o[:, 0:CC:2]
                        do = o[:, 1:CC:2]
                        out_t = out_pool.tile([P, 4 * CCH], f32)
                        eng2 = nc.gpsimd if stage2_split else nc.vector
                        nc.vector.tensor_add(out_t[:, 0 * CCH : 1 * CCH], se, so)  # cA
                        eng2.tensor_add(out_t[:, 1 * CCH : 2 * CCH], de, do)  # cH
                        nc.vector.tensor_sub(out_t[:, 2 * CCH : 3 * CCH], se, so)  # cV
                        eng2.tensor_sub(out_t[:, 3 * CCH : 4 * CCH], de, do)  # cD
                        if mode != "compute":
                            getattr(nc, store_engine).dma_start(
                                out=outr[rb, cc],
                                in_=out_t.rearrange("p (s c) -> p s c", s=4),
                            )

    nc.compile()
    return nc


def get_nc():
    if "nc" not in _CACHE:
        _CACHE["nc"] = _build_nc()
    return _CACHE["nc"]


def kernel(x: np.ndarray) -> np.ndarray:
    from concourse.bass_utils import run_bass_kernel_spmd

    x = np.ascontiguousarray(np.asarray(x, dtype=np.float32))
    assert x.shape == (H, W), x.shape
    nc = get_nc()
    in_maps = [{"x": x[i * HC : (i + 1) * HC]} for i in range(NCORES)]
    res = run_bass_kernel_spmd(nc, in_maps, core_ids=list(range(NCORES)))
    full = np.empty((4, H // 2, W // 2), dtype=np.float32)
    for i in range(NCORES):
        full[:, i * (HC // 2) : (i + 1) * (HC // 2), :] = res.results[i]["out"]
    return full



# revision 21
# speedup vs baseline: 1.0230x; 1.0230x over previous
"""2D Haar DWT (pywt.dwt2 'haar') on Trainium2, sharded across 8 NeuronCores.

Full input x: [8192, 8192] f32. Output: [4, 4096, 4096] f32 (cA, cH, cV, cD).

Sharding: row-wise. Core i handles rows [1024*i, 1024*(i+1)), producing output
rows [512*i, 512*(i+1)) of every subband. 2x2 haar blocks never cross the
chunk boundary, so no halo exchange.

Per-core dataflow (default "fullstore" layout; per 256-row block, 4 per core):
  - one 8MB DMA loads the block's 128 even rows into the left half and 128 odd
    rows into the right half of a [128, 2*8192] SBUF tile -- the row
    deinterleave is free in the DMA access pattern and every descriptor moves
    32KB contiguous
  - ScalarE: e *= 0.5 in place
  - VectorE scalar_tensor_tensor: d = (o * -0.5) + e = 0.5*(even-odd), into e
  - VectorE tensor_add:           s = d + o         = 0.5*(even+odd), into o
  - VectorE tensor_add/sub on stride-2 column views finish the butterfly,
    writing full-width subband pairs (cA|cH then cV|cD) into two alternating
    [128, 2*4096] tiles:
      cA = s[2j]+s[2j+1], cH = d[2j]+d[2j+1], cV = s[2j]-s[2j+1], cD = d[2j]-d[2j+1]
  - two 4MB DMAs store the pairs with 16KB-contiguous descriptors, issued on
    the ScalarE HWDGE ring so stores never head-of-line-block loads on the SP
    ring

The kernel is DMA-bound: 64MB of HBM traffic per core at ~340-350 GB/s
effective (~193us measured via repeat-NEFF differencing; the pure-roofline
floor at 358 GB/s/core is ~179us). Larger descriptor runs measurably beat
finer-grained pipelining on real HW (chunked 8KB-run variant: 243us).
"""

import numpy as np

H = 8192
W = 8192
NCORES = 8
HC = H // NCORES  # 1024 rows per core
P = 128  # partitions
C = 4096  # column chunk width (input cols per tile)
CH = C // 2  # output cols per tile per subband
N_RB = HC // (2 * P)  # 4 row blocks (each covers 256 input rows)
N_CC = W // C  # 2 column chunks

_CACHE: dict = {}


def _build_nc(
    repeat: int = 1,
    store_engine: str = "scalar",
    in_bufs: int = 2,
    s_bufs: int = 4,
    out_bufs: int = 1,
    scale_engine: str = "scalar",
    chunk: int = 4096,
    load_engine: str = "sync",
    mode: str = "full",
    layout: str = "fullstore",
    stage2_split: bool = False,
    combined_load: bool = True,
    shared_out: bool = False,
    reclaim: bool = False,
    store_engine2: str | None = None,
    load_engine2: str | None = None,
):
    import concourse.bacc as bacc
    import concourse.mybir as mybir
    from concourse.tile import TileContext

    f32 = mybir.dt.float32
    Alu = mybir.AluOpType

    if reclaim:
        # No SWDGE DMAs are used (loads/stores are HWDGE, scale is on ACT),
        # so drop the 16KB DynamicDMAScratch carveout; also release the 128B
        # of preallocated const tiles (their memsets run pre-barrier, before
        # any pool tile is written, so overlapping them is ordered-safe).
        nc = bacc.Bacc(
            "TRN2", target_bir_lowering=False, debug=False,
            dynamic_dma_scratch_size=0,
        )
        nc.sbuf_base = 0
    else:
        nc = bacc.Bacc("TRN2", target_bir_lowering=False, debug=False)
    x = nc.dram_tensor("x", [HC, W], f32, kind="ExternalInput").ap()
    out = nc.dram_tensor("out", [4, HC // 2, W // 2], f32, kind="ExternalOutput").ap()

    CC = chunk
    CCH = CC // 2
    n_cc = W // CC
    # x rows: rb*256 + p*2 + eo ; cols: cc*CC + c
    xr = x.rearrange("(rb p eo) (cc c) -> rb cc p eo c", p=P, eo=2, cc=n_cc)
    # out: subband s, row rb*128 + p, col cc*CCH + c
    outr = out.rearrange("s (rb p) (cc c) -> rb cc p s c", p=P, c=CCH)

    if layout == "mono":
        # One shared pool, 3 slots of [128, 2W] (64KB/partition, 192KB total).
        # Per row block: in_t (one 8MB load, 32KB runs) and out_t (one 8MB
        # store, 16KB runs) come from the same tag, so the allocator rotates
        # load(rb+1) / compute(rb) / store(rb-1) across the three slots.
        CW = W // 2
        xr3 = x.rearrange("(rb p eo) w -> rb p eo w", p=P, eo=2)
        outm = out.rearrange("s (rb p) c -> rb p s c", p=P)
        with TileContext(nc) as tc:
            with tc.tile_pool(name="u", bufs=in_bufs) as pool:
                for _rep in range(repeat):
                    for rb in range(N_RB):
                        in_t = pool.tile([P, 2 * W], f32, tag="u")
                        getattr(nc, load_engine).dma_start(
                            out=in_t.rearrange("p (eo w) -> p eo w", eo=2),
                            in_=xr3[rb],
                        )
                        e_t = in_t[:, 0:W]
                        o_t = in_t[:, W : 2 * W]
                        if scale_engine == "scalar":
                            nc.scalar.mul(e_t, e_t, 0.5)
                        else:
                            nc.gpsimd.tensor_scalar_mul(e_t, e_t, 0.5)
                        nc.vector.scalar_tensor_tensor(
                            out=e_t, in0=o_t, scalar=-0.5, in1=e_t,
                            op0=Alu.mult, op1=Alu.add,
                        )
                        nc.vector.tensor_add(o_t, e_t, o_t)
                        d_t, s_t2 = e_t, o_t
                        se = s_t2[:, 0:W:2]
                        so = s_t2[:, 1:W:2]
                        de = d_t[:, 0:W:2]
                        do = d_t[:, 1:W:2]
                        out_t = pool.tile([P, 2 * W], f32, tag="u")
                        nc.vector.tensor_add(out_t[:, 0 * CW : 1 * CW], se, so)  # cA
                        nc.vector.tensor_add(out_t[:, 1 * CW : 2 * CW], de, do)  # cH
                        nc.vector.tensor_sub(out_t[:, 2 * CW : 3 * CW], se, so)  # cV
                        nc.vector.tensor_sub(out_t[:, 3 * CW : 4 * CW], de, do)  # cD
                        getattr(nc, store_engine).dma_start(
                            out=outm[rb],
                            in_=out_t.rearrange("p (s c) -> p s c", s=4),
                        )
        nc.compile()
        return nc

    if layout == "fullstore":
        # Full-width everything: one combined [128, 2W] load per row block
        # (32KB runs), full-width stage-2, and per-subband-pair full-width
        # stores (16KB runs). Output double-buffered via two alternating
        # 2-subband pools so SBUF fits: 128 + 32 + 32 = 192KB.
        CW = W // 2
        xr3 = x.rearrange("(rb p eo) w -> rb p eo w", p=P, eo=2)
        xr2f = x.rearrange("(rb p eo) w -> rb eo p w", p=P, eo=2)
        # out dims for a 2-subband store: [p, s(2), c(W/2)]
        outp = out.rearrange("(sp s) (rb p) c -> rb sp p s c", s=2, p=P)
        with TileContext(nc) as tc:
            with (
                tc.tile_pool(name="inp", bufs=in_bufs) as in_pool,
                tc.tile_pool(name="onp", bufs=in_bufs) as o_pool_f,
                tc.tile_pool(name="outa", bufs=out_bufs) as pool_a,
                tc.tile_pool(name="outb", bufs=out_bufs) as pool_b,
            ):
                for _rep in range(repeat):
                    for rb in range(N_RB):
                        if combined_load:
                            in_t = in_pool.tile([P, 2 * W], f32)
                            getattr(nc, load_engine).dma_start(
                                out=in_t.rearrange("p (eo w) -> p eo w", eo=2),
                                in_=xr3[rb],
                            )
                            e_t = in_t[:, 0:W]
                            o_t = in_t[:, W : 2 * W]
                        else:
                            e_t = in_pool.tile([P, W], f32)
                            o_t = o_pool_f.tile([P, W], f32)
                            getattr(nc, load_engine).dma_start(out=e_t, in_=xr2f[rb, 0])
                            getattr(nc, load_engine2 or load_engine).dma_start(
                                out=o_t, in_=xr2f[rb, 1]
                            )
                        if scale_engine == "scalar":
                            nc.scalar.mul(e_t, e_t, 0.5)
                        else:
                            nc.gpsimd.tensor_scalar_mul(e_t, e_t, 0.5)
                        # d = -0.5*o + 0.5*e (into e half); s = d + o (into o half)
                        nc.vector.scalar_tensor_tensor(
                            out=e_t, in0=o_t, scalar=-0.5, in1=e_t,
                            op0=Alu.mult, op1=Alu.add,
                        )
                        nc.vector.tensor_add(o_t, e_t, o_t)
                        d_t, s_t2 = e_t, o_t
                        se = s_t2[:, 0:W:2]
                        so = s_t2[:, 1:W:2]
                        de = d_t[:, 0:W:2]
                        do = d_t[:, 1:W:2]
                        # pair 0: cA | cH ; pair 1: cV | cD
                        if shared_out:
                            t_a = pool_a.tile([P, 2 * CW], f32, tag="ot")
                            t_b = pool_a.tile([P, 2 * CW], f32, tag="ot")
                        else:
                            t_a = pool_a.tile([P, 2 * CW], f32)
                            t_b = pool_b.tile([P, 2 * CW], f32)
                        nc.vector.tensor_add(t_a[:, 0:CW], se, so)  # cA
                        nc.vector.tensor_add(t_a[:, CW : 2 * CW], de, do)  # cH
                        getattr(nc, store_engine).dma_start(
                            out=outp[rb, 0],
                            in_=t_a.rearrange("p (s c) -> p s c", s=2),
                        )
                        nc.vector.tensor_sub(t_b[:, 0:CW], se, so)  # cV
                        nc.vector.tensor_sub(t_b[:, CW : 2 * CW], de, do)  # cD
                        getattr(nc, store_engine2 or store_engine).dma_start(
                            out=outp[rb, 1],
                            in_=t_b.rearrange("p (s c) -> p s c", s=2),
                        )
        nc.compile()
        return nc

    if layout == "fullrow":
        # Full-width loads (32KB contiguous per partition-row), stage-1 in
        # place (d over e, s over o), half-width stores.
        NSC = W // 2 // CCH  # store chunks per row block
        xr2 = x.rearrange("(rb p eo) w -> rb eo p w", p=P, eo=2)
        xr3 = x.rearrange("(rb p eo) w -> rb p eo w", p=P, eo=2)
        with TileContext(nc) as tc:
            with (
                tc.tile_pool(name="ep", bufs=in_bufs) as e_pool,
                tc.tile_pool(name="op", bufs=in_bufs) as o_pool,
                tc.tile_pool(name="outp", bufs=out_bufs) as out_pool,
            ):
                for _rep in range(repeat):
                    for rb in range(N_RB):
                        if combined_load:
                            in_t = e_pool.tile([P, 2 * W], f32)
                            getattr(nc, load_engine).dma_start(
                                out=in_t.rearrange("p (eo w) -> p eo w", eo=2),
                                in_=xr3[rb],
                            )
                            e_t = in_t[:, 0:W]
                            o_t = in_t[:, W : 2 * W]
                        else:
                            e_t = e_pool.tile([P, W], f32)
                            o_t = o_pool.tile([P, W], f32)
                            getattr(nc, load_engine).dma_start(out=e_t, in_=xr2[rb, 0])
                            getattr(nc, load_engine).dma_start(out=o_t, in_=xr2[rb, 1])
                        if mode != "dma":
                            if scale_engine == "scalar":
                                nc.scalar.mul(e_t, e_t, 0.5)
                            else:
                                nc.gpsimd.tensor_scalar_mul(e_t, e_t, 0.5)
                            # d = -0.5*o + 0.5*e  (into e_t)
                            nc.vector.scalar_tensor_tensor(
                                out=e_t, in0=o_t, scalar=-0.5, in1=e_t,
                                op0=Alu.mult, op1=Alu.add,
                            )
                            # s = d + o = 0.5*e + 0.5*o  (into o_t)
                            nc.vector.tensor_add(o_t, e_t, o_t)
                        d_t, s_t2 = e_t, o_t
                        for sc in range(NSC):
                            lo = sc * 2 * CCH
                            hi = (sc + 1) * 2 * CCH
                            out_t = out_pool.tile([P, 4 * CCH], f32)
                            if mode != "dma":
                                se = s_t2[:, lo:hi:2]
                                so = s_t2[:, lo + 1 : hi : 2]
                                de = d_t[:, lo:hi:2]
                                do = d_t[:, lo + 1 : hi : 2]
                                eng2 = nc.gpsimd if stage2_split else nc.vector
                                nc.vector.tensor_add(out_t[:, 0 * CCH : 1 * CCH], se, so)
                                eng2.tensor_add(out_t[:, 1 * CCH : 2 * CCH], de, do)
                                nc.vector.tensor_sub(out_t[:, 2 * CCH : 3 * CCH], se, so)
                                eng2.tensor_sub(out_t[:, 3 * CCH : 4 * CCH], de, do)
                                src_ap = out_t.rearrange("p (s c) -> p s c", s=4)
                            else:
                                src_ap = e_t[:, 0 : 4 * CCH].rearrange(
                                    "p (s c) -> p s c", s=4
                                )
                            getattr(nc, store_engine).dma_start(
                                out=outr[rb, sc], in_=src_ap
                            )
        nc.compile()
        return nc

    with TileContext(nc) as tc:
        with (
            tc.tile_pool(name="inp", bufs=in_bufs) as in_pool,
            tc.tile_pool(name="sum", bufs=s_bufs) as s_pool,
            tc.tile_pool(name="outp", bufs=out_bufs) as out_pool,
        ):
            for _rep in range(repeat):
                for rb in range(N_RB):
                    for cc in range(n_cc):
                        in_t = in_pool.tile([P, 2 * CC], f32)
                        if mode != "compute":
                            getattr(nc, load_engine).dma_start(
                                out=in_t.rearrange("p (eo c) -> p eo c", eo=2),
                                in_=xr[rb, cc],
                            )
                        if mode == "dma":
                            getattr(nc, store_engine).dma_start(
                                out=outr[rb, cc],
                                in_=in_t[:, 0 : 4 * CCH].rearrange(
                                    "p (s c) -> p s c", s=4
                                ),
                            )
                            continue
                        e = in_t[:, 0:CC]
                        o = in_t[:, CC : 2 * CC]
                        # e <- 0.5*e (off VectorE: ScalarE or GpSimd)
                        if scale_engine == "scalar":
                            nc.scalar.mul(e, e, 0.5)
                        else:
                            nc.gpsimd.tensor_scalar_mul(e, e, 0.5)
                        s_t = s_pool.tile([P, CC], f32)
                        # s = 0.5*o + e(=0.5e)  ;  d = -0.5*o + e  (d in place over o)
                        nc.vector.scalar_tensor_tensor(
                            out=s_t, in0=o, scalar=0.5, in1=e, op0=Alu.mult, op1=Alu.add
                        )
                        nc.vector.scalar_tensor_tensor(
                            out=o, in0=o, scalar=-0.5, in1=e, op0=Alu.mult, op1=Alu.add
                        )
                        se = s_t[:, 0:CC:2]
                        so = s_t[:, 1:CC:2]
                        de = o[:, 0:CC:2]
                        do = o[:, 1:CC:2]
                        out_t = out_pool.tile([P, 4 * CCH], f32)
                        eng2 = nc.gpsimd if stage2_split else nc.vector
                        nc.vector.tensor_add(out_t[:, 0 * CCH : 1 * CCH], se, so)  # cA
                        eng2.tensor_add(out_t[:, 1 * CCH : 2 * CCH], de, do)  # cH
                        nc.vector.tensor_sub(out_t[:, 2 * CCH : 3 * CCH], se, so)  # cV
                        eng2.tensor_sub(out_t[:, 3 * CCH : 4 * CCH], de, do)  # cD
                        if mode != "compute":
                            getattr(nc, store_engine).dma_start(
                                out=outr[rb, cc],
                                in_=out_t.rearrange("p (s c) -> p s c", s=4),
                            )

    nc.compile()
    return nc


def get_nc():
    if "nc" not in _CACHE:
        _CACHE["nc"] = _build_nc()
    return _CACHE["nc"]


def kernel(x: np.ndarray) -> np.ndarray:
    from concourse.bass_utils import run_bass_kernel_spmd

    x = np.ascontiguousarray(np.asarray(x, dtype=np.float32))
    assert x.shape == (H, W), x.shape
    nc = get_nc()
    in_maps = [{"x": x[i * HC : (i + 1) * HC]} for i in range(NCORES)]
    res = run_bass_kernel_spmd(nc, in_maps, core_ids=list(range(NCORES)))
    full = np.empty((4, H // 2, W // 2), dtype=np.float32)
    for i in range(NCORES):
        full[:, i * (HC // 2) : (i + 1) * (HC // 2), :] = res.results[i]["out"]
    return full

